# revision 6
# baseline (speedup 1.0000x reference)
"""Causal multi-head attention with KV cache (steady state) on 8 TRN2 NeuronCores.

Problem shapes: B=8, S=512, D=512, H=8, dk=64, MAX_CACHE_LEN=1024, offset=1024.
Sharding: batch dim across the 8 cores (1 batch element per core), weights
replicated.

Per-core device kernel (all matmul compute in bf16, f32 accumulation):
  - host pre-transposes x, weights and the cached K so every matmul operand
    already has its contraction dim on partitions
  - Q^T, K^T projections in [feature, token] layout; V projection in
    [token, feature] layout (which is exactly the [key, dv] layout the
    attention output matmul wants)
  - scores computed transposed per head: S^T[key, q] = K @ Q^T; softmax
    numerator via ScalarE exp (no max subtraction: |s|/8 is tiny), causal
    masking via a triangular mask on the diagonal 128x128 block only plus
    trapezoid skipping of fully-masked regions
  - softmax denominators come for free from a ones-column appended to V
    (row 64 of the [65, q] attn@V accumulator)
  - normalization: 1/sum broadcast across partitions via a PE outer product
  - output projection back to [token, feature] with the bias folded in as a
    rank-1 matmul
"""

import numpy as np

B, S, D, H = 8, 512, 512, 8
DK = D // H  # 64
L = 1024  # MAX_CACHE_LEN
NC = 8  # cores
HC = L - S  # 512 cached keys actually used (last half of the cache)

_BUILT = None  # cached (nc, input names, output names)
LAST_RESULT = None  # BassKernelResults of the most recent run (for profiling)


def _split_multi_waits(nc):
    """The walrus build on this image rejects instructions with more than one
    embedded sync wait ("Too many sync wait commands"). Hoist all but one wait
    of every instruction into standalone InstEventSemaphore instructions right
    before it on the same engine queue (same semantics: per-engine FIFO)."""
    import concourse.mybir as mybir

    n = [0]
    for f in nc.m.functions:
        for b in f.blocks:
            insts = list(b.instructions)
            out = []
            changed = False
            for i in insts:
                si = i.sync_info
                waits = list(si.on_wait) if si and si.on_wait else []
                if len(waits) > 1:
                    changed = True
                    for w in waits[:-1]:
                        n[0] += 1
                        ev = mybir.InstEventSemaphore(
                            name=f"I-waitsplit-{n[0]}",
                            engine=i.engine,
                            ins=[],
                            outs=[],
                            sync_info=mybir.SyncInfo(on_wait=[w], on_update=[]),
                        )
                        out.append(ev)
                    i.sync_info = mybir.SyncInfo(
                        on_wait=[waits[-1]], on_update=list(si.on_update or [])
                    )
                out.append(i)
            if changed:
                b.instructions = out
    return n[0]


def _build_bass(split_waits=True):
    """Build the single-core Bass/Tile module (same program on all 8 cores)."""
    from contextlib import ExitStack

    import concourse.bass as bass
    import concourse.mybir as mybir
    import concourse.tile as tile
    from concourse.bass import ts

    bf16 = mybir.dt.bfloat16
    f32 = mybir.dt.float32
    Act = mybir.ActivationFunctionType

    nc = bass.Bass(trn_type="TRN2")

    # ---- DRAM I/O (per-core shapes) ----
    xT_d = nc.dram_tensor("xT", [4, 128, S], bf16, kind="ExternalInput")
    wqT_d = nc.dram_tensor("wqT", [4, 128, D], bf16, kind="ExternalInput")
    wkT_d = nc.dram_tensor("wkT", [4, 128, D], bf16, kind="ExternalInput")
    wvT_d = nc.dram_tensor("wvT", [4, 128, D], bf16, kind="ExternalInput")
    woT_d = nc.dram_tensor("woT", [4, 128, D], bf16, kind="ExternalInput")
    bqT_d = nc.dram_tensor("bqT", [128, 4], f32, kind="ExternalInput")
    bkT_d = nc.dram_tensor("bkT", [128, 4], f32, kind="ExternalInput")
    bv_d = nc.dram_tensor("bv", [1, D], bf16, kind="ExternalInput")
    bo_d = nc.dram_tensor("bo", [1, D], bf16, kind="ExternalInput")
    kcT_d = nc.dram_tensor("kcT", [128, 4, HC], bf16, kind="ExternalInput")
    vc_d = nc.dram_tensor("vc", [4, 128, H * (DK + 1)], bf16, kind="ExternalInput")
    mask_d = nc.dram_tensor("mask", [128, 128], bf16, kind="ExternalInput")

    out_d = nc.dram_tensor("out_o", [S, D], f32, kind="ExternalOutput")
    kn_d = nc.dram_tensor("k_new", [D, S], f32, kind="ExternalOutput")
    vn_d = nc.dram_tensor("v_new", [S, D], f32, kind="ExternalOutput")

    W = DK + 1  # 65: v columns per head incl. the ones column

    with ExitStack() as ctx:
        tc = ctx.enter_context(tile.TileContext(nc))
        const = ctx.enter_context(tc.tile_pool(name="const", bufs=1))
        act = ctx.enter_context(tc.tile_pool(name="act", bufs=3))
        psum = ctx.enter_context(
            tc.tile_pool(name="psum", bufs=2, space=bass.MemorySpace.PSUM)
        )

        # ---- resident loads ----
        xT = const.tile([128, 4, S], bf16)
        nc.sync.dma_start(xT[:], xT_d.rearrange("a p t -> p a t"))
        wqT = const.tile([128, 4, D], bf16)
        nc.sync.dma_start(wqT[:], wqT_d.rearrange("a p t -> p a t"))
        wkT = const.tile([128, 4, D], bf16)
        nc.sync.dma_start(wkT[:], wkT_d.rearrange("a p t -> p a t"))
        wvT = const.tile([128, 4, D], bf16)
        nc.sync.dma_start(wvT[:], wvT_d.rearrange("a p t -> p a t"))
        woT = const.tile([128, 4, D], bf16)
        nc.sync.dma_start(woT[:], woT_d.rearrange("a p t -> p a t"))
        bqT = const.tile([128, 4], f32)
        nc.sync.dma_start(bqT[:], bqT_d[:])
        bkT = const.tile([128, 4], f32)
        nc.sync.dma_start(bkT[:], bkT_d[:])
        bv = const.tile([1, D], bf16)
        nc.sync.dma_start(bv[:], bv_d[:])
        bo = const.tile([1, D], bf16)
        nc.sync.dma_start(bo[:], bo_d[:])
        kcT = const.tile([128, 4, HC], bf16)
        nc.sync.dma_start(kcT[:], kcT_d[:])
        vc = const.tile([128, 4, H * W], bf16)
        nc.sync.dma_start(vc[:], vc_d.rearrange("a p t -> p a t"))
        mask = const.tile([128, 128], bf16)
        nc.sync.dma_start(mask[:], mask_d[:])

        ones_row = const.tile([1, 128], bf16)
        nc.vector.memset(ones_row[:], 1.0)

        # ---- projections: Q^T, K^T in [o, t] layout ----
        qT = const.tile([128, 4, S], bf16)
        kT = const.tile([128, 4, S], bf16)
        for oj in range(4):
            q_ps = psum.tile([128, S], f32, tag="mm")
            for ki in range(4):
                nc.tensor.matmul(
                    q_ps[:],
                    wqT[:, ki, ts(oj, 128)],
                    xT[:, ki, :],
                    start=(ki == 0),
                    stop=(ki == 3),
                )
            nc.scalar.activation(
                qT[:, oj, :], q_ps[:], Act.Identity, bias=bqT[:, oj : oj + 1]
            )

            k_ps = psum.tile([128, S], f32, tag="mm")
            for ki in range(4):
                nc.tensor.matmul(
                    k_ps[:],
                    wkT[:, ki, ts(oj, 128)],
                    xT[:, ki, :],
                    start=(ki == 0),
                    stop=(ki == 3),
                )
            nc.scalar.activation(
                kT[:, oj, :], k_ps[:], Act.Identity, bias=bkT[:, oj : oj + 1]
            )
            kf = act.tile([128, S], f32, tag="kf")
            nc.vector.tensor_scalar_add(kf[:], k_ps[:], bkT[:, oj : oj + 1])
            nc.sync.dma_start(kn_d[ts(oj, 128), :], kf[:])

        # ---- V projection in [t, o] layout, bias folded via rank-1 matmul ----
        v_sb = const.tile([128, 4, H * W], bf16)
        for ti in range(4):
            v_ps = psum.tile([128, D], f32, tag="mm")
            for ki in range(4):
                nc.tensor.matmul(
                    v_ps[:],
                    xT[:, ki, ts(ti, 128)],
                    wvT[:, ki, :],
                    start=(ki == 0),
                    stop=False,
                )
            nc.tensor.matmul(v_ps[:], ones_row[:, 0:128], bv[:], start=False, stop=True)
            vf = act.tile([128, D], f32, tag="vf")
            nc.vector.tensor_copy(vf[:], v_ps[:])
            nc.sync.dma_start(vn_d[ts(ti, 128), :], vf[:])
            # pack into [key, h*65 + c] layout with a ones column per head
            vv = v_sb[:, ti, :].rearrange("p (h c) -> p h c", h=H)
            nc.vector.memset(vv[:, :, DK : DK + 1], 1.0)
            nc.vector.tensor_copy(
                vv[:, :, 0:DK], v_ps[:].rearrange("p (h c) -> p h c", h=H)
            )

        # ---- attention per head ----
        oT = const.tile([128, 4, S], bf16)  # normalized O^T, p = (h%2)*64 + d
        for h in range(H):
            hp, hl = h // 2, h % 2
            o_ps = psum.tile([W, S], f32, tag="o")
            for kt in range(8):
                if kt < 4:
                    lk = kcT[hl * 64 : hl * 64 + 64, hp, ts(kt, 128)]
                    qlo = 0
                else:
                    j = kt - 4
                    lk = kT[hl * 64 : hl * 64 + 64, hp, ts(j, 128)]
                    qlo = 128 * j
                s_ps = psum.tile([128, S], f32, tag="s")
                nc.tensor.matmul(
                    s_ps[:, qlo:S],
                    lk,
                    qT[hl * 64 : hl * 64 + 64, hp, qlo:S],
                    start=True,
                    stop=True,
                )
                p_sb = act.tile([128, S], bf16, tag="p")
                nc.scalar.activation(
                    p_sb[:, qlo:S], s_ps[:, qlo:S], Act.Exp, scale=float(DK**-0.5)
                )
                if kt >= 4:
                    nc.vector.tensor_mul(
                        p_sb[:, qlo : qlo + 128], p_sb[:, qlo : qlo + 128], mask[:]
                    )
                lv = vc[:, kt, h * W : (h + 1) * W] if kt < 4 else v_sb[
                    :, kt - 4, h * W : (h + 1) * W
                ]
                nc.tensor.matmul(
                    o_ps[:, qlo:S], lv, p_sb[:, qlo:S], start=(kt == 0), stop=(kt == 7)
                )
            # normalization: row DK of o_ps holds the softmax denominators
            rcp = act.tile([1, S], f32, tag="rcp")
            nc.vector.reciprocal(rcp[:], o_ps[DK : DK + 1, :])
            rcpb = act.tile([1, S], bf16, tag="rcpb")
            nc.scalar.copy(rcpb[:], rcp[:])
            bc_ps = psum.tile([64, S], f32, tag="bc", bufs=1)
            nc.tensor.matmul(bc_ps[:], ones_row[:, 0:64], rcpb[:], start=True, stop=True)
            bc_sb = act.tile([64, S], bf16, tag="bcs")
            nc.vector.tensor_copy(bc_sb[:], bc_ps[:])
            nc.vector.tensor_mul(
                oT[hl * 64 : hl * 64 + 64, hp, :], o_ps[0:DK, :], bc_sb[:]
            )

        # ---- output projection, bo folded via rank-1 matmul ----
        for ti in range(4):
            u_ps = psum.tile([128, D], f32, tag="mm")
            for dj in range(4):
                nc.tensor.matmul(
                    u_ps[:],
                    oT[:, dj, ts(ti, 128)],
                    woT[:, dj, :],
                    start=(dj == 0),
                    stop=False,
                )
            nc.tensor.matmul(u_ps[:], ones_row[:, 0:128], bo[:], start=False, stop=True)
            of = act.tile([128, D], f32, tag="of")
            nc.vector.tensor_copy(of[:], u_ps[:])
            nc.sync.dma_start(out_d[ts(ti, 128), :], of[:])

    if split_waits:
        _split_multi_waits(nc)

    in_names = [
        "xT", "wqT", "wkT", "wvT", "woT", "bqT", "bkT", "bv", "bo", "kcT", "vc",
        "mask",
    ]
    out_names = ["out_o", "k_new", "v_new"]
    return nc, in_names, out_names


def _get_built():
    global _BUILT
    if _BUILT is None:
        _BUILT = _build_bass()
    return _BUILT


def _host_inputs(x, prev_k, prev_v, Wq, bq, Wk, bk, Wv, bv, Wo, bo):
    """Build the 8 per-core input maps (host-side layout prep)."""
    bf16 = np.dtype("bfloat16") if hasattr(np, "bfloat16") else None
    import ml_dtypes

    bf = ml_dtypes.bfloat16

    def b16(a):
        return np.ascontiguousarray(a).astype(bf)

    # weights, replicated across cores
    wqT = b16(Wq.T.reshape(4, 128, D))
    wkT = b16(Wk.T.reshape(4, 128, D))
    wvT = b16(Wv.T.reshape(4, 128, D))
    woT = b16(Wo.T.reshape(4, 128, D))
    bqT = np.ascontiguousarray(bq.reshape(4, 128).T).astype(np.float32)
    bkT = np.ascontiguousarray(bk.reshape(4, 128).T).astype(np.float32)
    bvr = b16(bv.reshape(1, D))
    bor = b16(bo.reshape(1, D))
    maskt = np.triu(np.ones((128, 128), np.float32)).astype(bf)

    pk = prev_k[:, :, HC:, :]  # [B, H, 512, 64]
    pv = prev_v[:, :, HC:, :]

    # kcT: [128, 4, 512] with p = (h%2)*64 + d, j = h//2
    kcT = (
        pk.transpose(0, 1, 3, 2)  # [B, H, 64, 512]
        .reshape(B, 4, 2, DK, HC)
        .transpose(0, 2, 3, 1, 4)  # [B, 2, 64, 4, 512]
        .reshape(B, 128, 4, HC)
    )
    # vc: [4, 128, 8*65] with ones column per head
    tmp = pv.transpose(0, 2, 1, 3)  # [B, 512, H, 64]
    vc = np.concatenate([tmp, np.ones((B, HC, H, 1), np.float32)], axis=3)
    vc = vc.reshape(B, 4, 128, H * (DK + 1))

    in_maps = []
    for c in range(NC):
        xT = b16(x[c].T.reshape(4, 128, S))
        in_maps.append(
            {
                "xT": xT,
                "wqT": wqT,
                "wkT": wkT,
                "wvT": wvT,
                "woT": woT,
                "bqT": bqT,
                "bkT": bkT,
                "bv": bvr,
                "bo": bor,
                "kcT": b16(kcT[c]),
                "vc": b16(vc[c]),
                "mask": maskt,
            }
        )
    return in_maps


def kernel(x, prev_k, prev_v, Wq, bq, Wk, bk, Wv, bv, Wo, bo, offset):
    global LAST_RESULT
    import os

    x = np.asarray(x, np.float32)
    prev_k = np.asarray(prev_k, np.float32)
    prev_v = np.asarray(prev_v, np.float32)
    Wq, bq = np.asarray(Wq, np.float32), np.asarray(bq, np.float32)
    Wk, bk = np.asarray(Wk, np.float32), np.asarray(bk, np.float32)
    Wv, bv = np.asarray(Wv, np.float32), np.asarray(bv, np.float32)
    Wo, bo = np.asarray(Wo, np.float32), np.asarray(bo, np.float32)
    off = int(np.asarray(offset))
    assert off >= S, f"kernel hardcodes steady-state causal layout (offset={off})"

    from concourse import bass_utils

    nc, in_names, out_names = _get_built()
    in_maps = _host_inputs(x, prev_k, prev_v, Wq, bq, Wk, bk, Wv, bv, Wo, bo)

    trace = bool(int(os.environ.get("KERNEL_TRACE", "0")))
    res = bass_utils.run_bass_kernel_spmd(
        nc, in_maps, core_ids=list(range(NC)), trace=trace
    )
    LAST_RESULT = res

    out = np.empty((B, S, D), np.float32)
    k_full = np.empty((B, H, L, DK), np.float32)
    v_full = np.empty((B, H, L, DK), np.float32)
    k_full[:, :, :HC] = prev_k[:, :, HC:]
    v_full[:, :, :HC] = prev_v[:, :, HC:]
    for c in range(NC):
        r = res.results[c]
        out[c] = r["out_o"]
        k_full[c, :, HC:] = r["k_new"].reshape(H, DK, S).transpose(0, 2, 1)
        v_full[c, :, HC:] = r["v_new"].reshape(S, H, DK).transpose(1, 0, 2)

    new_offset = np.int32(min(off + S, L))
    return out, k_full, v_full, new_offset
